# revision 6
# baseline (speedup 1.0000x reference)
"""Megatron-style TP attention kernel for trn2 (8 NeuronCores).

Problem: LayerNorm -> fused QKV -> causal MHA -> fp16 output projection.
  B=2, S=2048, M=2048, H=16 heads, D=128.

Sharding: DP=2 over batch x TP=4 over heads. Core c handles batch c//4 and
heads 4*(c%4)..4*(c%4)+3. Each core computes its 4 heads' context, the TP
group AllGathers the (fp16) context, and each core then computes a disjoint
512-column slice of the output projection, so no all-reduce is needed.
The host reassembles the full [B,S,M] output from the 8 column slices.

All on-device layouts are "transposed" (contraction dim on partitions):
  xT [m, s], qkvwT [m, n], qT/kT [d, s] per head, v [s, d], ctxT [i, s].
LayerNorm runs in the transposed layout: column stats via ones-matmuls on
the PE, mean/rstd broadcast across partitions via gpsimd.
Softmax needs no max-subtraction (scores are tiny; masked lanes get exact
zeros via multiplicative masks after exp), and normalization is deferred to
the PSUM eviction of probs@V.
Matmuls use float32r (full PE rate at free dim >= 256); output projection
uses fp16 operands like the reference.
"""

import numpy as np

import concourse.bass as bass
import concourse.mybir as mybir
import concourse.tile as tile
from concourse import bacc
from concourse.bass_utils import run_bass_kernel_spmd

FP32 = mybir.dt.float32
FP32R = mybir.dt.float32r
FP16 = mybir.dt.float16

N_CORES = 8
B, S, M, H = 2, 2048, 2048, 16
D = M // H            # 128
TP = 4                # heads groups
DP = 2                # batch
HPC = H // TP         # 4 heads per core
NSL = HPC * D         # 512: per-core q/k/v column slice
EPS = 1e-5
P = 128
SC = 512              # s-chunk for phase 1
NCH = S // SC         # 4 chunks
MT = M // P           # 16 m-tiles
ST = S // P           # 16 s-tiles

# exp writes float32r tiles directly from the ACT engine; flip if the BIR
# verifier rejects ACT-produced fp32r.
ACT_F32R = True

_cached = {}


def build_program():
    nc = bacc.Bacc(
        "TRN2", target_bir_lowering=False, debug=False, num_devices=N_CORES
    )

    xT = nc.dram_tensor("xT", [M, S], FP32, kind="ExternalInput")
    wq = nc.dram_tensor("wq", [M, 3 * NSL], FP32, kind="ExternalInput")
    bqk = nc.dram_tensor("bqk", [P, 8], FP32, kind="ExternalInput")
    bv = nc.dram_tensor("bv", [P, HPC], FP32, kind="ExternalInput")
    owT = nc.dram_tensor("owT", [M, NSL], FP16, kind="ExternalInput")
    obr = nc.dram_tensor("obr", [1, NSL], FP32, kind="ExternalInput")
    cmask = nc.dram_tensor("cmask", [4, P, SC], FP32, kind="ExternalInput")
    ones = nc.dram_tensor("ones", [P, 1], FP32, kind="ExternalInput")
    out = nc.dram_tensor("out", [S, NSL], FP32, kind="ExternalOutput")

    xT_r = xT[:].bitcast(FP32R).rearrange("(mt p) s -> p mt s", p=P)
    wq_r = wq[:].bitcast(FP32R).rearrange("(mt p) n -> p mt n", p=P)

    with tile.TileContext(nc) as tc:
        with (
            tc.tile_pool(name="const", bufs=1) as const,
            tc.tile_pool(name="dram", bufs=1, space="DRAM") as dram,
            tc.tile_pool(name="qkres", bufs=1) as qkres,
        ):
            # constants
            ones_r = const.tile([P, 1], FP32R)
            nc.sync.dma_start(out=ones_r[:], in_=ones[:].bitcast(FP32R))
            bqk_sb = const.tile([P, 8], FP32)
            nc.sync.dma_start(out=bqk_sb[:], in_=bqk[:])
            bv_sb = const.tile([P, HPC], FP32)
            nc.sync.dma_start(out=bv_sb[:], in_=bv[:])
            mask_sb = const.tile([P, 4, SC], FP32)
            nc.sync.dma_start(out=mask_sb[:], in_=cmask[:].rearrange("j p q -> p j q"))
            obr_sb = const.tile([1, NSL], FP32)
            nc.sync.dma_start(out=obr_sb[:], in_=obr[:])
            obr_b = const.tile([P, NSL], FP32)
            nc.gpsimd.partition_broadcast(obr_b[:], obr_sb[:])
            eps_t = const.tile([1, 1], FP32)
            nc.vector.memset(eps_t[:], EPS)
            # fp16 output weights, resident: [p, it, j]
            owT_sb = const.tile([P, MT, NSL], FP16)
            nc.sync.dma_start(
                out=owT_sb[:], in_=owT[:].rearrange("(it p) j -> p it j", p=P)
            )

            # v, resident in SBUF for the attention phase: [p, st, hpc*D]
            v_sb = qkres.tile([P, ST, NSL], FP32R)
            # q/k staged through DRAM: idx 0..3 = qT per head, 4..7 = kT
            qk_dram = dram.tile([8, P, S], FP32)
            cc_in = dram.tile([NSL, S], FP16)
            cc_out = dram.tile([TP * NSL, S], FP16)

            # ---------------- Phase 1: LayerNorm + QKV projection ----------
            with (
                tc.tile_pool(name="panel", bufs=2) as panel,
                tc.tile_pool(name="wpool", bufs=2) as wpool,
                tc.tile_pool(name="wvpool", bufs=4) as wvpool,
                tc.tile_pool(name="sqpool", bufs=2) as sqpool,
                tc.tile_pool(name="rows", bufs=2) as rows,
                tc.tile_pool(name="bcast", bufs=2) as bcast,
                tc.tile_pool(name="qkev", bufs=3) as qkev,
                tc.tile_pool(name="psum1", bufs=2, space="PSUM") as psum1,
                tc.tile_pool(name="psumv", bufs=1, space="PSUM") as psumv,
                tc.tile_pool(name="psums", bufs=1, space="PSUM") as psums,
            ):
                for sc in range(NCH):
                    ssl = slice(sc * SC, (sc + 1) * SC)
                    xp = panel.tile([P, MT, SC], FP32R, tag="xp")
                    nc.sync.dma_start(out=xp[:], in_=xT_r[:, :, ssl])

                    # column stats over m via ones-matmuls
                    ssum = psums.tile([1, SC], FP32, tag="ssum")
                    ssum2 = psums.tile([1, SC], FP32, tag="ssum2")
                    for mt in range(MT):
                        sq_t = sqpool.tile([P, SC], FP32R, tag="sq")
                        nc.vector.tensor_mul(
                            out=sq_t[:],
                            in0=xp[:, mt, :].bitcast(FP32),
                            in1=xp[:, mt, :].bitcast(FP32),
                        )
                        nc.tensor.matmul(
                            ssum[:], ones_r[:], xp[:, mt, :],
                            start=(mt == 0), stop=(mt == MT - 1),
                        )
                        nc.tensor.matmul(
                            ssum2[:], ones_r[:], sq_t[:],
                            start=(mt == 0), stop=(mt == MT - 1),
                        )

                    mu_row = rows.tile([1, SC], FP32, tag="mu")
                    nc.vector.tensor_scalar_mul(
                        out=mu_row[:], in0=ssum[:], scalar1=1.0 / M
                    )
                    msq_row = rows.tile([1, SC], FP32, tag="msq")
                    nc.vector.tensor_scalar_mul(
                        out=msq_row[:], in0=ssum2[:], scalar1=1.0 / M
                    )
                    var_row = rows.tile([1, SC], FP32, tag="var")
                    nc.vector.tensor_mul(out=var_row[:], in0=mu_row[:], in1=mu_row[:])
                    nc.vector.tensor_sub(out=var_row[:], in0=msq_row[:], in1=var_row[:])
                    std_row = rows.tile([1, SC], FP32, tag="std")
                    nc.scalar.activation(
                        out=std_row[:], in_=var_row[:],
                        func=mybir.ActivationFunctionType.Sqrt,
                        bias=eps_t[:],
                    )
                    rstd_row = rows.tile([1, SC], FP32, tag="rstd")
                    nc.vector.reciprocal(out=rstd_row[:], in_=std_row[:])

                    mu_b = bcast.tile([P, SC], FP32, tag="mub")
                    nc.gpsimd.partition_broadcast(mu_b[:], mu_row[:])
                    rstd_b = bcast.tile([P, SC], FP32, tag="rstdb")
                    nc.gpsimd.partition_broadcast(rstd_b[:], rstd_row[:])

                    # xn^T in place: (x - mu) * rstd  (ln g/b folded into W on host)
                    for mt in range(MT):
                        nc.vector.tensor_sub(
                            out=xp[:, mt, :],
                            in0=xp[:, mt, :].bitcast(FP32),
                            in1=mu_b[:],
                        )
                        nc.vector.tensor_mul(
                            out=xp[:, mt, :],
                            in0=xp[:, mt, :].bitcast(FP32),
                            in1=rstd_b[:],
                        )

                    # q/k projections: out [n-tile, s-chunk]
                    for nt in range(8):
                        w_t = wpool.tile([P, MT, P], FP32R, tag="w")
                        nc.sync.dma_start(
                            out=w_t[:], in_=wq_r[:, :, nt * P : (nt + 1) * P]
                        )
                        qkp = psum1.tile([P, SC], FP32, tag="qkp")
                        for mt in range(MT):
                            nc.tensor.matmul(
                                qkp[:], w_t[:, mt, :], xp[:, mt, :],
                                start=(mt == 0), stop=(mt == MT - 1),
                            )
                        qk_ev = qkev.tile([P, SC], FP32R, tag="qkev")
                        nc.vector.tensor_scalar_add(
                            out=qk_ev[:], in0=qkp[:], scalar1=bqk_sb[:, nt : nt + 1]
                        )
                        nc.sync.dma_start(
                            out=qk_dram[nt, :, ssl].bitcast(FP32R), in_=qk_ev[:]
                        )

                    # v projection in natural [s, (h d)] layout
                    vps = [
                        psumv.tile([P, NSL], FP32, tag=f"vp{st}", name=f"vp{st}")
                        for st in range(SC // P)
                    ]
                    for mt in range(MT):
                        wv_t = wvpool.tile([P, NSL], FP32R, tag="wv")
                        nc.sync.dma_start(
                            out=wv_t[:], in_=wq_r[:, mt, 2 * NSL : 3 * NSL]
                        )
                        for st in range(SC // P):
                            nc.tensor.matmul(
                                vps[st][:],
                                xp[:, mt, st * P : (st + 1) * P],
                                wv_t[:],
                                start=(mt == 0), stop=(mt == MT - 1),
                            )
                    for st in range(SC // P):
                        nc.vector.tensor_copy(
                            out=v_sb[:, sc * (SC // P) + st, :], in_=vps[st][:]
                        )

            # ---------------- Phase 2: causal attention per (head, q-chunk) --
            with (
                tc.tile_pool(name="ktp", bufs=2) as ktp,
                tc.tile_pool(name="qtp", bufs=2) as qtp,
                tc.tile_pool(name="expp", bufs=4) as expp,
                tc.tile_pool(name="exptmp", bufs=3) as exptmp,
                tc.tile_pool(name="rnorm", bufs=4) as rnorm,
                tc.tile_pool(name="ctxf", bufs=4) as ctxf,
                tc.tile_pool(name="psst", bufs=2, space="PSUM") as psst,
                tc.tile_pool(name="psctx", bufs=2, space="PSUM") as psctx,
                tc.tile_pool(name="psr", bufs=2, space="PSUM") as psr,
            ):
                for h in range(HPC):
                    for qc in range(NCH):
                        kmax = 4 * (qc + 1)  # k-tiles 0..kmax-1
                        qsl = slice(qc * SC, (qc + 1) * SC)
                        kT_t = ktp.tile([P, S], FP32R, tag="kt")
                        nc.sync.dma_start(
                            out=kT_t[:, : kmax * P],
                            in_=qk_dram[4 + h, :, : kmax * P].bitcast(FP32R),
                        )
                        qT_t = qtp.tile([P, SC], FP32R, tag="qt")
                        nc.sync.dma_start(
                            out=qT_t[:], in_=qk_dram[h, :, qsl].bitcast(FP32R)
                        )

                        ctxp = psctx.tile([P, SC], FP32, tag="ctxp")
                        rp = psr.tile([1, SC], FP32, tag="rp")
                        for kt in range(kmax):
                            stp = psst.tile([P, SC], FP32, tag="stp")
                            nc.tensor.matmul(
                                stp[:],
                                kT_t[:, kt * P : (kt + 1) * P],
                                qT_t[:],
                                start=True, stop=True,
                            )
                            expT = expp.tile([P, SC], FP32R, tag="expT")
                            jdiag = kt - 4 * qc
                            if jdiag >= 0:
                                et = exptmp.tile([P, SC], FP32, tag="et")
                                nc.scalar.activation(
                                    out=et[:], in_=stp[:],
                                    func=mybir.ActivationFunctionType.Exp,
                                )
                                nc.vector.tensor_mul(
                                    out=expT[:], in0=et[:], in1=mask_sb[:, jdiag, :]
                                )
                            elif ACT_F32R:
                                nc.scalar.activation(
                                    out=expT[:], in_=stp[:],
                                    func=mybir.ActivationFunctionType.Exp,
                                )
                            else:
                                et = exptmp.tile([P, SC], FP32, tag="et")
                                nc.scalar.activation(
                                    out=et[:], in_=stp[:],
                                    func=mybir.ActivationFunctionType.Exp,
                                )
                                nc.vector.tensor_copy(out=expT[:], in_=et[:])
                            nc.tensor.matmul(
                                ctxp[:],
                                v_sb[:, kt, h * P : (h + 1) * P],
                                expT[:],
                                start=(kt == 0), stop=(kt == kmax - 1),
                            )
                            nc.tensor.matmul(
                                rp[:], ones_r[:], expT[:],
                                start=(kt == 0), stop=(kt == kmax - 1),
                            )

                        rinv = rnorm.tile([1, SC], FP32, tag="rinv")
                        nc.vector.reciprocal(out=rinv[:], in_=rp[:])
                        rinv_b = rnorm.tile([P, SC], FP32, tag="rinvb")
                        nc.gpsimd.partition_broadcast(rinv_b[:], rinv[:])
                        ctx_t = ctxf.tile([P, SC], FP32, tag="ctxt")
                        nc.vector.tensor_mul(out=ctx_t[:], in0=ctxp[:], in1=rinv_b[:])
                        ctx16 = ctxf.tile([P, SC], FP16, tag="ctx16")
                        nc.vector.tensor_scalar_add(
                            out=ctx16[:], in0=ctx_t[:], scalar1=bv_sb[:, h : h + 1]
                        )
                        nc.gpsimd.dma_start(
                            out=cc_in[h * P : (h + 1) * P, qsl], in_=ctx16[:]
                        )

                nc.gpsimd.collective_compute(
                    "AllGather",
                    mybir.AluOpType.bypass,
                    replica_groups=[[0, 1, 2, 3], [4, 5, 6, 7]],
                    ins=[cc_in.opt()],
                    outs=[cc_out.opt()],
                )

            # ---------------- Phase 3: fp16 output projection ----------------
            with (
                tc.tile_pool(name="cst", bufs=3) as cstp,
                tc.tile_pool(name="outev", bufs=3) as outev,
                tc.tile_pool(name="psout", bufs=3, space="PSUM") as psout,
            ):
                cc_out_r = cc_out[:].rearrange("(it p) s -> p it s", p=P)
                for st in range(ST):
                    cst = cstp.tile([P, MT, P], FP16, tag="cst")
                    nc.gpsimd.dma_start(
                        out=cst[:], in_=cc_out_r[:, :, st * P : (st + 1) * P]
                    )
                    op = psout.tile([P, NSL], FP32, tag="op")
                    for it in range(MT):
                        nc.tensor.matmul(
                            op[:], cst[:, it, :], owT_sb[:, it, :],
                            start=(it == 0), stop=(it == MT - 1),
                        )
                    o_ev = outev.tile([P, NSL], FP32, tag="oev")
                    nc.vector.tensor_add(out=o_ev[:], in0=op[:], in1=obr_b[:])
                    nc.sync.dma_start(
                        out=out[st * P : (st + 1) * P, :], in_=o_ev[:]
                    )

    nc.compile()
    return nc


def _prep_inputs(x, ln_g, ln_b, qkvw, qkvb, ow, ob):
    x = np.asarray(x, dtype=np.float32)
    ln_g = np.asarray(ln_g, dtype=np.float32)
    ln_b = np.asarray(ln_b, dtype=np.float32)
    qkvw = np.asarray(qkvw, dtype=np.float32)
    qkvb = np.asarray(qkvb, dtype=np.float32)
    ow = np.asarray(ow, dtype=np.float16)
    ob = np.asarray(ob, dtype=np.float16)

    # fold LayerNorm affine into the QKV weights/bias:
    #   qkv = (xn*g + b) @ W^T + qb = xn @ (W*g)^T + (qb + W @ b)
    qkvwT = np.ascontiguousarray(qkvw.T)  # [M, 3M]
    qkvwT *= ln_g[:, None]
    qkvb_f = qkvb + qkvw @ ln_b

    owT = np.ascontiguousarray(ow.T)  # [M, M] fp16

    kp = np.arange(P)[:, None]
    qf = np.arange(SC)[None, :]
    cmask = np.stack(
        [(qf >= P * j + kp).astype(np.float32) for j in range(4)], axis=0
    )
    ones = np.ones([P, 1], np.float32)

    in_maps = []
    for c in range(N_CORES):
        b, g = divmod(c, TP)
        ns = slice(NSL * g, NSL * (g + 1))
        wq_c = np.ascontiguousarray(
            np.concatenate(
                [qkvwT[:, ns], qkvwT[:, M:][:, ns], qkvwT[:, 2 * M :][:, ns]],
                axis=1,
            )
        )
        bq = qkvb_f[ns].reshape(HPC, P).T
        bk = qkvb_f[M:][ns].reshape(HPC, P).T
        bqk_c = np.ascontiguousarray(np.concatenate([bq, bk], axis=1))
        bv_c = np.ascontiguousarray(qkvb_f[2 * M :][ns].reshape(HPC, P).T)
        in_maps.append(
            {
                "xT": np.ascontiguousarray(x[b].T),
                "wq": wq_c,
                "bqk": bqk_c.astype(np.float32),
                "bv": bv_c.astype(np.float32),
                "owT": np.ascontiguousarray(owT[:, ns]),
                "obr": np.ascontiguousarray(
                    ob[ns].astype(np.float32)[None, :]
                ),
                "cmask": cmask,
                "ones": ones,
            }
        )
    return in_maps


def kernel(x, ln_g, ln_b, qkvw, qkvb, ow, ob, _trace=False, _results=None):
    if "nc" not in _cached:
        _cached["nc"] = build_program()
    nc = _cached["nc"]
    in_maps = _prep_inputs(x, ln_g, ln_b, qkvw, qkvb, ow, ob)
    res = run_bass_kernel_spmd(
        nc, in_maps, list(range(N_CORES)), trace=_trace
    )
    if _results is not None:
        _results.append(res)
    full = np.empty([B, S, M], np.float32)
    for c in range(N_CORES):
        b, g = divmod(c, TP)
        full[b, :, NSL * g : NSL * (g + 1)] = res.results[c]["out"]
    return full


# revision 14
# speedup vs baseline: 1.0836x; 1.0836x over previous
"""Megatron-style TP attention kernel for trn2 (8 NeuronCores).

Problem: LayerNorm -> fused QKV -> causal MHA -> fp16 output projection.
  B=2, S=2048, M=2048, H=16 heads, D=128.

Sharding: DP=2 over batch x TP=4 over heads. Core c handles batch c//4 and
heads 4*(c%4)..4*(c%4)+3. Each core computes its 4 heads' context, the TP
group AllGathers the (fp16) context, and each core then computes a disjoint
512-column slice of the output projection, so no all-reduce is needed.
The host reassembles the full [B,S,M] output from the 8 column slices.

All on-device layouts are "transposed" (contraction dim on partitions):
  xT [m, s], qkvwT [m, n], qT/kT [d, s] per head, v [s, d], ctxT [i, s].
LayerNorm runs in the transposed layout: column stats via ones-matmuls on
the PE, mean/rstd broadcast across partitions via gpsimd.
Softmax needs no max-subtraction (scores are tiny; masked lanes get exact
zeros via multiplicative masks after exp), and normalization is deferred to
the PSUM eviction of probs@V.
Matmuls use float32r (full PE rate at free dim >= 256); output projection
uses fp16 operands like the reference.
"""

import numpy as np

import concourse.bass as bass
import concourse.mybir as mybir
import concourse.tile as tile
from concourse import bacc
from concourse.bass_utils import run_bass_kernel_spmd

FP32 = mybir.dt.float32
FP32R = mybir.dt.float32r
FP16 = mybir.dt.float16

N_CORES = 8
B, S, M, H = 2, 2048, 2048, 16
D = M // H            # 128
TP = 4                # heads groups
DP = 2                # batch
HPC = H // TP         # 4 heads per core
NSL = HPC * D         # 512: per-core q/k/v column slice
EPS = 1e-5
P = 128
SC = 512              # s-chunk for phase 1
NCH = S // SC         # 4 chunks
MT = M // P           # 16 m-tiles
ST = S // P           # 16 s-tiles

# exp writes float32r tiles directly from the ACT engine; flip if the BIR
# verifier rejects ACT-produced fp32r.
ACT_F32R = True

_cached = {}


def build_program():
    nc = bacc.Bacc(
        "TRN2",
        target_bir_lowering=False,
        debug=False,
        num_devices=N_CORES,
        enable_partition_id=True,
    )

    xT = nc.dram_tensor("xT", [M, S], FP32, kind="ExternalInput")
    wq = nc.dram_tensor("wq", [M, 3 * NSL], FP32, kind="ExternalInput")
    bqk = nc.dram_tensor("bqk", [P, 8], FP32, kind="ExternalInput")
    bv = nc.dram_tensor("bv", [P, HPC], FP32, kind="ExternalInput")
    owT = nc.dram_tensor("owT", [M, NSL], FP16, kind="ExternalInput")
    obr = nc.dram_tensor("obr", [1, NSL], FP32, kind="ExternalInput")
    cmask = nc.dram_tensor("cmask", [4, P, SC], FP32, kind="ExternalInput")
    ones = nc.dram_tensor("ones", [P, 1], FP32, kind="ExternalInput")
    out = nc.dram_tensor("out", [S, NSL], FP32, kind="ExternalOutput")

    xT_r = xT[:].bitcast(FP32R).rearrange("(mt p) s -> p mt s", p=P)
    wq_r = wq[:].bitcast(FP32R).rearrange("(mt p) n -> p mt n", p=P)

    with tile.TileContext(nc) as tc:
        with (
            tc.tile_pool(name="const", bufs=1) as const,
            tc.tile_pool(name="dram", bufs=1, space="DRAM") as dram,
            tc.tile_pool(name="qkres", bufs=1) as qkres,
        ):
            # constants
            ones_r = const.tile([P, 1], FP32R)
            nc.sync.dma_start(out=ones_r[:], in_=ones[:].bitcast(FP32R))
            bqk_sb = const.tile([P, 8], FP32)
            nc.sync.dma_start(out=bqk_sb[:], in_=bqk[:])
            bv_sb = const.tile([P, HPC], FP32)
            nc.sync.dma_start(out=bv_sb[:], in_=bv[:])
            mask_sb = const.tile([P, 4, SC], FP32)
            nc.sync.dma_start(out=mask_sb[:], in_=cmask[:].rearrange("j p q -> p j q"))
            obr_sb = const.tile([1, NSL], FP32)
            nc.sync.dma_start(out=obr_sb[:], in_=obr[:])
            obr_b = const.tile([P, NSL], FP32)
            nc.gpsimd.partition_broadcast(obr_b[:], obr_sb[:])
            eps_t = const.tile([1, 1], FP32)
            nc.vector.memset(eps_t[:], EPS)
            # fp16 output weights, resident: [p, it, j]
            owT_sb = const.tile([P, MT, NSL], FP16)
            nc.sync.dma_start(
                out=owT_sb[:], in_=owT[:].rearrange("(it p) j -> p it j", p=P)
            )

            # v, resident in SBUF for the attention phase: [p, st, hpc*D]
            v_sb = qkres.tile([P, ST, NSL], FP32R)
            # q/k staged through DRAM: idx 0..3 = qT per head, 4..7 = kT
            qk_dram = dram.tile([8, P, S], FP32)
            cc_in = dram.tile([NSL, S], FP16)
            cc_out = dram.tile([N_CORES * NSL, S], FP16, addr_space="Shared")

            # ---------------- Phase 1: LayerNorm + QKV projection ----------
            with (
                tc.tile_pool(name="panel", bufs=2) as panel,
                tc.tile_pool(name="wpool", bufs=2) as wpool,
                tc.tile_pool(name="wvpool", bufs=4) as wvpool,
                tc.tile_pool(name="sqpool", bufs=2) as sqpool,
                tc.tile_pool(name="rows", bufs=2) as rows,
                tc.tile_pool(name="bcast", bufs=2) as bcast,
                tc.tile_pool(name="qkev", bufs=3) as qkev,
                tc.tile_pool(name="psum1", bufs=2, space="PSUM") as psum1,
                tc.tile_pool(name="psumv", bufs=1, space="PSUM") as psumv,
                tc.tile_pool(name="psums", bufs=1, space="PSUM") as psums,
            ):
                for sc in range(NCH):
                    ssl = slice(sc * SC, (sc + 1) * SC)
                    # per-m-tile panel tiles so downstream deps are fine-grained
                    xps = []
                    for mt in range(MT):
                        xp_t = panel.tile(
                            [P, SC], FP32R, tag=f"xp{mt}", name=f"xp{mt}"
                        )
                        nc.sync.dma_start(out=xp_t[:], in_=xT_r[:, mt, ssl])
                        xps.append(xp_t)

                    # column stats over m via ones-matmuls
                    ssum = psums.tile([1, SC], FP32, tag="ssum")
                    ssum2 = psums.tile([1, SC], FP32, tag="ssum2")
                    for mt in range(MT):
                        sq_t = sqpool.tile([P, SC], FP32R, tag="sq")
                        nc.vector.tensor_mul(
                            out=sq_t[:],
                            in0=xps[mt][:].bitcast(FP32),
                            in1=xps[mt][:].bitcast(FP32),
                        )
                        nc.tensor.matmul(
                            ssum[:], ones_r[:], xps[mt][:],
                            start=(mt == 0), stop=(mt == MT - 1),
                        )
                        nc.tensor.matmul(
                            ssum2[:], ones_r[:], sq_t[:],
                            start=(mt == 0), stop=(mt == MT - 1),
                        )

                    mu_row = rows.tile([1, SC], FP32, tag="mu")
                    nc.vector.tensor_scalar_mul(
                        out=mu_row[:], in0=ssum[:], scalar1=1.0 / M
                    )
                    msq_row = rows.tile([1, SC], FP32, tag="msq")
                    nc.vector.tensor_scalar_mul(
                        out=msq_row[:], in0=ssum2[:], scalar1=1.0 / M
                    )
                    var_row = rows.tile([1, SC], FP32, tag="var")
                    nc.vector.tensor_mul(out=var_row[:], in0=mu_row[:], in1=mu_row[:])
                    nc.vector.tensor_sub(out=var_row[:], in0=msq_row[:], in1=var_row[:])
                    std_row = rows.tile([1, SC], FP32, tag="std")
                    nc.scalar.activation(
                        out=std_row[:], in_=var_row[:],
                        func=mybir.ActivationFunctionType.Sqrt,
                        bias=eps_t[:],
                    )
                    rstd_row = rows.tile([1, SC], FP32, tag="rstd")
                    nc.vector.reciprocal(out=rstd_row[:], in_=std_row[:])

                    mu_b = bcast.tile([P, SC], FP32, tag="mub")
                    nc.gpsimd.partition_broadcast(mu_b[:], mu_row[:])
                    rstd_b = bcast.tile([P, SC], FP32, tag="rstdb")
                    nc.gpsimd.partition_broadcast(rstd_b[:], rstd_row[:])

                    # xn^T in place: (x - mu) * rstd  (ln g/b folded into W on host)
                    for mt in range(MT):
                        nc.vector.tensor_sub(
                            out=xps[mt][:],
                            in0=xps[mt][:].bitcast(FP32),
                            in1=mu_b[:],
                        )
                        nc.vector.tensor_mul(
                            out=xps[mt][:],
                            in0=xps[mt][:].bitcast(FP32),
                            in1=rstd_b[:],
                        )

                    # q/k projections: out [n-tile, s-chunk]
                    for nt in range(8):
                        w_t = wpool.tile([P, MT, P], FP32R, tag="w")
                        nc.sync.dma_start(
                            out=w_t[:], in_=wq_r[:, :, nt * P : (nt + 1) * P]
                        )
                        qkp = psum1.tile([P, SC], FP32, tag="qkp")
                        for mt in range(MT):
                            nc.tensor.matmul(
                                qkp[:], w_t[:, mt, :], xps[mt][:],
                                start=(mt == 0), stop=(mt == MT - 1),
                            )
                        qk_ev = qkev.tile([P, SC], FP32R, tag="qkev")
                        nc.vector.tensor_scalar_add(
                            out=qk_ev[:], in0=qkp[:], scalar1=bqk_sb[:, nt : nt + 1]
                        )
                        nc.sync.dma_start(
                            out=qk_dram[nt, :, ssl].bitcast(FP32R), in_=qk_ev[:]
                        )

                    # v projection in natural [s, (h d)] layout
                    vps = [
                        psumv.tile([P, NSL], FP32, tag=f"vp{st}", name=f"vp{st}")
                        for st in range(SC // P)
                    ]
                    for mt in range(MT):
                        wv_t = wvpool.tile([P, NSL], FP32R, tag="wv")
                        nc.sync.dma_start(
                            out=wv_t[:], in_=wq_r[:, mt, 2 * NSL : 3 * NSL]
                        )
                        for st in range(SC // P):
                            nc.tensor.matmul(
                                vps[st][:],
                                xps[mt][:, st * P : (st + 1) * P],
                                wv_t[:],
                                start=(mt == 0), stop=(mt == MT - 1),
                            )
                    for st in range(SC // P):
                        nc.vector.tensor_copy(
                            out=v_sb[:, sc * (SC // P) + st, :], in_=vps[st][:]
                        )

            # ---------------- Phase 2: causal attention per (head, q-chunk) --
            with (
                tc.tile_pool(name="ktp", bufs=2) as ktp,
                tc.tile_pool(name="qtp", bufs=2) as qtp,
                tc.tile_pool(name="expp", bufs=4) as expp,
                tc.tile_pool(name="exptmp", bufs=3) as exptmp,
                tc.tile_pool(name="rnorm", bufs=4) as rnorm,
                tc.tile_pool(name="ctxf", bufs=4) as ctxf,
                tc.tile_pool(name="psst", bufs=2, space="PSUM") as psst,
                tc.tile_pool(name="psctx", bufs=2, space="PSUM") as psctx,
                tc.tile_pool(name="psr", bufs=2, space="PSUM") as psr,
            ):
                for h in range(HPC):
                    for qc in range(NCH):
                        kmax = 4 * (qc + 1)  # k-tiles 0..kmax-1
                        qsl = slice(qc * SC, (qc + 1) * SC)
                        kT_t = ktp.tile([P, S], FP32R, tag="kt")
                        nc.sync.dma_start(
                            out=kT_t[:, : kmax * P],
                            in_=qk_dram[4 + h, :, : kmax * P].bitcast(FP32R),
                        )
                        qT_t = qtp.tile([P, SC], FP32R, tag="qt")
                        nc.sync.dma_start(
                            out=qT_t[:], in_=qk_dram[h, :, qsl].bitcast(FP32R)
                        )

                        ctxp = psctx.tile([P, SC], FP32, tag="ctxp")
                        rp = psr.tile([1, SC], FP32, tag="rp")
                        for kt in range(kmax):
                            stp = psst.tile([P, SC], FP32, tag="stp")
                            nc.tensor.matmul(
                                stp[:],
                                kT_t[:, kt * P : (kt + 1) * P],
                                qT_t[:],
                                start=True, stop=True,
                            )
                            expT = expp.tile([P, SC], FP32R, tag="expT")
                            jdiag = kt - 4 * qc
                            if jdiag >= 0:
                                et = exptmp.tile([P, SC], FP32, tag="et")
                                nc.scalar.activation(
                                    out=et[:], in_=stp[:],
                                    func=mybir.ActivationFunctionType.Exp,
                                )
                                nc.vector.tensor_mul(
                                    out=expT[:], in0=et[:], in1=mask_sb[:, jdiag, :]
                                )
                            elif ACT_F32R:
                                nc.scalar.activation(
                                    out=expT[:], in_=stp[:],
                                    func=mybir.ActivationFunctionType.Exp,
                                )
                            else:
                                et = exptmp.tile([P, SC], FP32, tag="et")
                                nc.scalar.activation(
                                    out=et[:], in_=stp[:],
                                    func=mybir.ActivationFunctionType.Exp,
                                )
                                nc.vector.tensor_copy(out=expT[:], in_=et[:])
                            nc.tensor.matmul(
                                ctxp[:],
                                v_sb[:, kt, h * P : (h + 1) * P],
                                expT[:],
                                start=(kt == 0), stop=(kt == kmax - 1),
                            )
                            nc.tensor.matmul(
                                rp[:], ones_r[:], expT[:],
                                start=(kt == 0), stop=(kt == kmax - 1),
                            )

                        rinv = rnorm.tile([1, SC], FP32, tag="rinv")
                        nc.vector.reciprocal(out=rinv[:], in_=rp[:])
                        rinv_b = rnorm.tile([P, SC], FP32, tag="rinvb")
                        nc.gpsimd.partition_broadcast(rinv_b[:], rinv[:])
                        ctx_t = ctxf.tile([P, SC], FP32, tag="ctxt")
                        nc.vector.tensor_mul(out=ctx_t[:], in0=ctxp[:], in1=rinv_b[:])
                        ctx16 = ctxf.tile([P, SC], FP16, tag="ctx16")
                        nc.vector.tensor_scalar_add(
                            out=ctx16[:], in0=ctx_t[:], scalar1=bv_sb[:, h : h + 1]
                        )
                        nc.gpsimd.dma_start(
                            out=cc_in[h * P : (h + 1) * P, qsl], in_=ctx16[:]
                        )

                # 8-rank AllGather (the 4-rank grouped version runs a slow
                # fold_n=2 ring at ~60 GB/s; the 8-rank algorithm is ~6x
                # faster and each core just reads its batch's half below).
                nc.gpsimd.collective_compute(
                    "AllGather",
                    mybir.AluOpType.bypass,
                    replica_groups=[[0, 1, 2, 3, 4, 5, 6, 7]],
                    ins=[cc_in.opt()],
                    outs=[cc_out.opt()],
                )

            # ---------------- Phase 3: fp16 output projection ----------------
            with (
                tc.tile_pool(name="cst", bufs=3) as cstp,
                tc.tile_pool(name="outev", bufs=3) as outev,
                tc.tile_pool(name="psout", bufs=3, space="PSUM") as psout,
            ):
                # this core's batch half of the gathered context: rows
                # [2048*bh, 2048*(bh+1)) of cc_out, bh = rank // 4
                bh = nc.gpsimd.partition_id() // TP
                cc_out_r = cc_out[:].rearrange("(b it p) s -> p b it s", b=DP, p=P)
                for st in range(ST):
                    cst = cstp.tile([P, MT, P], FP16, tag="cst")
                    nc.gpsimd.dma_start(
                        out=cst[:],
                        in_=cc_out_r[
                            :, bass.ds(bh, 1), :, st * P : (st + 1) * P
                        ].rearrange("p b it s -> p (b it) s"),
                    )
                    op = psout.tile([P, NSL], FP32, tag="op")
                    for it in range(MT):
                        nc.tensor.matmul(
                            op[:], cst[:, it, :], owT_sb[:, it, :],
                            start=(it == 0), stop=(it == MT - 1),
                        )
                    o_ev = outev.tile([P, NSL], FP32, tag="oev")
                    nc.vector.tensor_add(out=o_ev[:], in0=op[:], in1=obr_b[:])
                    nc.sync.dma_start(
                        out=out[st * P : (st + 1) * P, :], in_=o_ev[:]
                    )

    nc.compile()
    return nc


def _prep_inputs(x, ln_g, ln_b, qkvw, qkvb, ow, ob):
    x = np.asarray(x, dtype=np.float32)
    ln_g = np.asarray(ln_g, dtype=np.float32)
    ln_b = np.asarray(ln_b, dtype=np.float32)
    qkvw = np.asarray(qkvw, dtype=np.float32)
    qkvb = np.asarray(qkvb, dtype=np.float32)
    ow = np.asarray(ow, dtype=np.float16)
    ob = np.asarray(ob, dtype=np.float16)

    # fold LayerNorm affine into the QKV weights/bias:
    #   qkv = (xn*g + b) @ W^T + qb = xn @ (W*g)^T + (qb + W @ b)
    qkvwT = np.ascontiguousarray(qkvw.T)  # [M, 3M]
    qkvwT *= ln_g[:, None]
    qkvb_f = qkvb + qkvw @ ln_b

    owT = np.ascontiguousarray(ow.T)  # [M, M] fp16

    kp = np.arange(P)[:, None]
    qf = np.arange(SC)[None, :]
    cmask = np.stack(
        [(qf >= P * j + kp).astype(np.float32) for j in range(4)], axis=0
    )
    ones = np.ones([P, 1], np.float32)

    in_maps = []
    for c in range(N_CORES):
        b, g = divmod(c, TP)
        ns = slice(NSL * g, NSL * (g + 1))
        wq_c = np.ascontiguousarray(
            np.concatenate(
                [qkvwT[:, ns], qkvwT[:, M:][:, ns], qkvwT[:, 2 * M :][:, ns]],
                axis=1,
            )
        )
        bq = qkvb_f[ns].reshape(HPC, P).T
        bk = qkvb_f[M:][ns].reshape(HPC, P).T
        bqk_c = np.ascontiguousarray(np.concatenate([bq, bk], axis=1))
        bv_c = np.ascontiguousarray(qkvb_f[2 * M :][ns].reshape(HPC, P).T)
        in_maps.append(
            {
                "xT": np.ascontiguousarray(x[b].T),
                "wq": wq_c,
                "bqk": bqk_c.astype(np.float32),
                "bv": bv_c.astype(np.float32),
                "owT": np.ascontiguousarray(owT[:, ns]),
                "obr": np.ascontiguousarray(
                    ob[ns].astype(np.float32)[None, :]
                ),
                "cmask": cmask,
                "ones": ones,
            }
        )
    return in_maps


def kernel(x, ln_g, ln_b, qkvw, qkvb, ow, ob, _trace=False, _results=None):
    if "nc" not in _cached:
        _cached["nc"] = build_program()
    nc = _cached["nc"]
    in_maps = _prep_inputs(x, ln_g, ln_b, qkvw, qkvb, ow, ob)
    res = run_bass_kernel_spmd(
        nc, in_maps, list(range(N_CORES)), trace=_trace
    )
    if _results is not None:
        _results.append(res)
    full = np.empty([B, S, M], np.float32)
    for c in range(N_CORES):
        b, g = divmod(c, TP)
        full[b, :, NSL * g : NSL * (g + 1)] = res.results[c]["out"]
    return full
